# revision 37
# baseline (speedup 1.0000x reference)
"""Trainium2 Bass kernel for nn_MultiHeadAttention_55894704390646.

Multi-head causal attention, B=2, S=2048, E=1024, H=16 heads, D=64.
Sharding: data-parallel over batch (2 groups) x tensor-parallel over heads
(4 heads per core). Each core computes a partial output-projection result
(row-split Wo); the host sums the 4 partials per batch and adds the bias.

v3 design (all-bf16):
  - every matmul operand is bf16 (PSUM accumulation stays fp32): same
    1 cycle/row as fp32r but half the DMA traffic and SBUF bandwidth, and
    DVE 2x modes for the mask multiplies. End-to-end error ~4e-3 vs the
    2e-2 gate.
  - HW constraint discovered on the way: a matmul's PSUM output must
    start on a 2KB bank boundary (CoreSim accepts unaligned outputs;
    hardware dies). So scores/ctx keep one PSUM bank per head.
  - attention at 512-query chunks, scores transposed [keys, queries];
    exp skips the fully-masked strip of diagonal tiles (partial-width
    ACT + gpsimd memset of the zero strip), recovering most of the
    fine-causal ACT savings; softmax denominator comes from a ones
    column appended to v (M=65 AV), no max subtraction needed.
  - emission interleaves QKV-projection and output-projection matmul
    groups as filler between attention tile-groups (skew-1 score
    prefetch), so the in-order PE queue stays fed and the PE p-state
    clock stays high. Startup DMA is fine-grained (per-ko weight tiles
    interleaved with x chunk 0) so the first projection group starts
    after ~200KB instead of ~4MB.
  - ACT (scalar engine) runs only EXP + the tiny denominator-row
    copies; PSUM evacuations run on the vector engine, reciprocal
    broadcasts on gpsimd.
"""

import os
import sys

if "/opt/trn_rl_repo" not in sys.path:
    sys.path.insert(0, "/opt/trn_rl_repo")

import numpy as np
import ml_dtypes

import concourse.bass as bass
from concourse import bacc
import concourse.mybir as mybir
import concourse.tile as tile
from concourse.bass_utils import run_bass_kernel_spmd

B, S, E, H, D = 2, 2048, 1024, 16, 64
N_CORES = 8
DP = 2                 # batch groups
TP = 4                 # cores per batch group
HL = H // TP           # local heads per core = 4
DL = HL * D            # local head dims = 256
P = 128
NKO = E // P           # contraction blocks over E = 8
CH = 512               # token chunk (projections and attention)
NCH = S // CH          # chunks = 4
NTB = S // P           # 128-token blocks = 16
NPAIR = HL // 2        # head pairs = 2
NEO = E // CH          # output feature chunks of 512 = 2
NKB = CH // P          # k-blocks per chunk = 4
SKEW = 1               # score-group prefetch depth

f32 = mybir.dt.float32
bf16 = mybir.dt.bfloat16
EXP = mybir.ActivationFunctionType.Exp

_NC_CACHE = None


def _build_nc():
    nc = bacc.Bacc("TRN2", target_bir_lowering=False, debug=False)

    xT = nc.dram_tensor("xT", (E, S), bf16, kind="ExternalInput")
    wqT = nc.dram_tensor("wqT", (E, DL), bf16, kind="ExternalInput")
    wkT = nc.dram_tensor("wkT", (E, DL), bf16, kind="ExternalInput")
    wvT = nc.dram_tensor("wvT", (E, DL), bf16, kind="ExternalInput")
    woT = nc.dram_tensor("woT", (DL, E), bf16, kind="ExternalInput")
    out = nc.dram_tensor("out", (S, E), bf16, kind="ExternalOutput")

    with tile.TileContext(nc) as tc:
        with (
            nc.allow_low_precision(reason="bf16 matmuls; validated 4e-3 rel err"),
            tc.tile_pool(name="big", bufs=1) as big,
            tc.tile_pool(name="pt", bufs=4) as ptp,
            tc.tile_pool(name="work", bufs=3) as work,
            tc.tile_pool(name="osb", bufs=3) as osb,
            tc.tile_pool(name="ps", bufs=2, space="PSUM") as ps,
            tc.tile_pool(name="ps_s0", bufs=2, space="PSUM") as ps_s0,
            tc.tile_pool(name="ps_s1", bufs=2, space="PSUM") as ps_s1,
            tc.tile_pool(name="ps_c0", bufs=1, space="PSUM") as ps_c0,
            tc.tile_pool(name="ps_c1", bufs=1, space="PSUM") as ps_c1,
        ):
            # ---------------- DMA loads (fine-grained, startup-ordered) ----
            wq_sb = [None] * NKO
            wk_sb = [None] * NKO
            wv_sb = [None] * NKO
            x_sb = [[None] * NCH for _ in range(NKO)]
            for ko in range(NKO):
                wq_sb[ko] = big.tile([P, DL], bf16, tag=f"wq{ko}", name=f"wq{ko}")
                nc.sync.dma_start(wq_sb[ko][:], wqT[ko * P : (ko + 1) * P, :])
                x_sb[ko][0] = big.tile([P, CH], bf16, tag=f"x{ko}_0", name=f"x{ko}_0")
                nc.sync.dma_start(x_sb[ko][0][:], xT[ko * P : (ko + 1) * P, 0:CH])
            for ko in range(NKO):
                wk_sb[ko] = big.tile([P, DL], bf16, tag=f"wk{ko}", name=f"wk{ko}")
                nc.sync.dma_start(wk_sb[ko][:], wkT[ko * P : (ko + 1) * P, :])
            for ko in range(NKO):
                wv_sb[ko] = big.tile([P, DL], bf16, tag=f"wv{ko}", name=f"wv{ko}")
                nc.sync.dma_start(wv_sb[ko][:], wvT[ko * P : (ko + 1) * P, :])
            for c in range(1, NCH):
                for ko in range(NKO):
                    x_sb[ko][c] = big.tile(
                        [P, CH], bf16, tag=f"x{ko}_{c}", name=f"x{ko}_{c}"
                    )
                    nc.sync.dma_start(
                        x_sb[ko][c][:], xT[ko * P : (ko + 1) * P, c * CH : (c + 1) * CH]
                    )
                if c == 1:
                    wo_sb = big.tile([P, NPAIR, E], bf16, tag="wo")
                    nc.sync.dma_start(
                        wo_sb[:], woT[:].rearrange("(pr p) e -> p pr e", p=P)
                    )

            # causal mask for the 128-wide diagonal strip: tri[k, j] = k <= j
            tri = big.tile([P, P], bf16, tag="tri")
            nc.gpsimd.memset(tri[:], 1.0)
            nc.gpsimd.affine_select(
                out=tri[:],
                in_=tri[:],
                compare_op=mybir.AluOpType.is_ge,
                fill=0.0,
                base=0,
                pattern=[[1, P]],
                channel_multiplier=-1,
            )

            ones_stage = big.tile([P, HL], bf16, tag="ones_stage")
            nc.gpsimd.memset(ones_stage[:], 1.0)

            # persistent activations
            qT = [[None] * NCH for _ in range(NPAIR)]
            kT = [[None] * NCH for _ in range(NPAIR)]
            for pr in range(NPAIR):
                for c in range(NCH):
                    qT[pr][c] = big.tile([P, CH], bf16, tag=f"qT{pr}{c}", name=f"qT{pr}{c}")
                    kT[pr][c] = big.tile([P, CH], bf16, tag=f"kT{pr}{c}", name=f"kT{pr}{c}")
            v_tb = []
            for tb in range(NTB):
                vt = big.tile([P, HL, D + 2], bf16, tag=f"v{tb}", name=f"v{tb}")
                nc.vector.tensor_copy(vt[:, :, D], ones_stage[:, :])
                v_tb.append(vt)
            ctx_t = []
            for c in range(NCH):
                ctx_t.append(
                    big.tile([P, NPAIR, CH], bf16, tag=f"ctx{c}", name=f"ctx{c}")
                )

            # ---------------- emission helpers ----------------------------
            def emit_qk_group(c, pr, is_k):
                w_sb, dst = (wk_sb, kT) if is_k else (wq_sb, qT)
                pp = ps.tile([P, CH], f32, tag="mm", name=f"pp{c}{pr}{is_k}")
                for ko in range(NKO):
                    nc.tensor.matmul(
                        pp[:],
                        w_sb[ko][:, pr * P : (pr + 1) * P],
                        x_sb[ko][c][:],
                        start=(ko == 0),
                        stop=(ko == NKO - 1),
                    )
                nc.vector.tensor_copy(dst[pr][c][:], pp[:])

            def emit_v_tb(tb):
                pv_full = ps.tile([P, CH], f32, tag="mm", name=f"pv{tb}")
                pv = pv_full[:, 0:DL]
                c, j = tb // NKB, tb % NKB
                for ko in range(NKO):
                    nc.tensor.matmul(
                        pv[:],
                        x_sb[ko][c][:, j * P : (j + 1) * P],
                        wv_sb[ko][:],
                        start=(ko == 0),
                        stop=(ko == NKO - 1),
                    )
                nc.vector.tensor_copy(
                    v_tb[tb][:, :, 0:D], pv[:].rearrange("p (h d) -> p h d", h=HL)
                )

            def emit_scores(pr, J, I):
                """Scores + exp (+diag mask) for k-block I vs query chunk J.
                Per-head PSUM banks (matmul outs must be 2KB-aligned).
                Returns the bf16 pT tile [128, 2, 512]."""
                kc, ks = I // NKB, (I % NKB) * P
                di = I - NKB * J  # >=0 on the diagonal group of blocks
                lo = max(0, di) * P  # queries below lo are fully masked
                s0 = ps_s0.tile([P, CH], f32, tag="s0", name=f"s0_{pr}{J}{I}")
                s1 = ps_s1.tile([P, CH], f32, tag="s1", name=f"s1_{pr}{J}{I}")
                pT = ptp.tile([P, 2, CH], bf16, tag="pT", name=f"pT{pr}{J}{I}")
                for h, sh in ((0, s0), (1, s1)):
                    nc.tensor.matmul(
                        sh[:],
                        kT[pr][kc][h * D : (h + 1) * D, ks : ks + P],
                        qT[pr][J][h * D : (h + 1) * D, :],
                        start=True,
                        stop=True,
                    )
                if lo:
                    nc.gpsimd.memset(pT[:, :, 0:lo], 0.0)
                for h, sh in ((0, s0), (1, s1)):
                    nc.scalar.activation(
                        pT[:, h, lo:CH], sh[:, lo:CH], EXP, scale=0.125
                    )
                    if di >= 0:
                        nc.vector.tensor_tensor(
                            pT[:, h, lo : lo + P],
                            pT[:, h, lo : lo + P],
                            tri[:],
                            mybir.AluOpType.mult,
                        )
                return pT

            def emit_av(pr, J, I, pT, cx0, cx1):
                nI = NKB * (J + 1)
                last = I == nI - 1
                for h, cx in ((0, cx0), (1, cx1)):
                    nc.tensor.matmul(
                        cx[:], v_tb[I][:, 2 * pr + h, 0 : D + 1], pT[:, h, :],
                        start=(I == 0), stop=last,
                    )

            def emit_normalize(pr, J, cx0, cx1):
                """ctx_t[J][0:64 | 64:128, pr, :] = ctx/den for the pair.
                Each head's full 65 PSUM rows (ctx + den) are evacuated by
                one DVE copy, so the banks free after two DVE ops — the
                recip chain then runs entirely off PSUM. The reciprocal is
                computed in place at partition 64 (no cross-partition copy)
                and broadcast from there. h1 goes first: its path carries
                the extra SBUF-shift DMA latency (split across 4 queues)."""
                cu = work.tile([D, 2, CH], f32, tag="cu", name=f"cu{pr}{J}")
                nc.vector.tensor_copy(cu[:, 0, :], cx0[0:D, :])
                nc.vector.tensor_copy(cu[:, 1, :], cx1[0:D, :])
                dn = work.tile([1, 2, CH], f32, tag="dn", name=f"dn{pr}{J}")
                nc.scalar.copy(dn[:, 0, :], cx0[D : D + 1, :])
                nc.scalar.copy(dn[:, 1, :], cx1[D : D + 1, :])
                recip = work.tile([1, 2, CH], f32, tag="rc", name=f"rc{pr}{J}")
                nc.vector.reciprocal_approx_fast(recip[:], dn[:])
                dnb = work.tile([D, 2, CH], f32, tag="dnb", name=f"dnb{pr}{J}")
                nc.gpsimd.partition_broadcast(dnb[:, 0, :], recip[:, 0, :])
                nc.gpsimd.partition_broadcast(dnb[:, 1, :], recip[:, 1, :])
                # h1 first: its path has the extra SBUF-shift DMA latency
                tmp = work.tile([D, CH], bf16, tag="tmp", name=f"tmp{pr}{J}")
                nc.vector.tensor_tensor(
                    tmp[:], cu[:, 1, :], dnb[:, 1, :], mybir.AluOpType.mult
                )
                if J == NCH - 1:
                    # tail: spread the shift across queues so the final
                    # out-projections aren't gated on one 64KB transfer
                    for j in range(NKB):
                        nc.sync.dma_start(
                            ctx_t[J][D:P, pr, j * P : (j + 1) * P],
                            tmp[:, j * P : (j + 1) * P],
                        )
                else:
                    nc.sync.dma_start(ctx_t[J][D:P, pr, :], tmp[:])
                nc.vector.tensor_tensor(
                    ctx_t[J][0:D, pr, :], cu[:, 0, :], dnb[:, 0, :],
                    mybir.AluOpType.mult,
                )

            def emit_out(tb, ec):
                J, j = tb // NKB, tb % NKB
                o_ps = ps.tile([P, CH], f32, tag="mm", name=f"o{tb}{ec}")
                for pr in range(NPAIR):
                    nc.tensor.matmul(
                        o_ps[:],
                        ctx_t[J][:, pr, j * P : (j + 1) * P],
                        wo_sb[:, pr, ec * CH : (ec + 1) * CH],
                        start=(pr == 0),
                        stop=(pr == NPAIR - 1),
                    )
                o_sb = osb.tile([P, CH], bf16, tag="o_sb", name=f"osb{tb}{ec}")
                nc.vector.tensor_copy(o_sb[:], o_ps[:])
                if tb >= NTB - NKB:
                    # tail: halve the last store latency via two queues
                    half = CH // 2
                    for j in range(2):
                        nc.sync.dma_start(
                            out[
                                tb * P : (tb + 1) * P,
                                ec * CH + j * half : ec * CH + (j + 1) * half,
                            ],
                            o_sb[:, j * half : (j + 1) * half],
                        )
                else:
                    nc.sync.dma_start(
                        out[tb * P : (tb + 1) * P, ec * CH : (ec + 1) * CH], o_sb[:]
                    )

            # ---------------- schedule -------------------------------------
            # q-groups first: the wk DMAs land after wq, so the k-groups
            # get the extra slack of two full projection groups
            for pr in range(NPAIR):
                emit_qk_group(0, pr, False)
            for pr in range(NPAIR):
                emit_qk_group(0, pr, True)
            for tb in range(NKB):
                emit_v_tb(tb)

            units = [(pr, J) for J in range(NCH) for pr in range(NPAIR)]
            glist = []
            for pr, J in units:
                for I in range(NKB * (J + 1)):
                    glist.append((pr, J, I, I == NKB * (J + 1) - 1))

            def win_start(Jw):
                g = 0
                for pr, J in units:
                    if J == Jw:
                        return g
                    g += NKB * (J + 1)
                return len(glist)

            gidx = 0
            unit_end = {}
            for pr, J in units:
                gidx += NKB * (J + 1)
                unit_end[(pr, J)] = gidx

            fillers = []
            for c in range(1, NCH):
                avail = win_start(c - 1)
                for pr in range(NPAIR):
                    fillers.append((avail, 4096, lambda c=c, pr=pr: emit_qk_group(c, pr, False)))
                    fillers.append((avail, 4096, lambda c=c, pr=pr: emit_qk_group(c, pr, True)))
                for tb in range(NKB * c, NKB * c + NKB):
                    fillers.append((avail, 2048, lambda tb=tb: emit_v_tb(tb)))
            for J in range(NCH - 1):  # last chunk's outs go in the tail
                avail = unit_end[(1, J)] + SKEW
                for tb in range(NKB * J, NKB * (J + 1)):
                    for ec in range(NEO):
                        fillers.append((avail, 1024, lambda tb=tb, ec=ec: emit_out(tb, ec)))
            fillers.sort(key=lambda f: f[0])

            ctx_of_unit = {}
            fi = 0
            budget = 0.0
            pending = []
            for gi, (pr, J, I, last) in enumerate(glist):
                if I == 0:
                    cx0 = ps_c0.tile([D + 1, CH], f32, tag="c0", name=f"cx0_{pr}{J}")
                    cx1 = ps_c1.tile([D + 1, CH], f32, tag="c1", name=f"cx1_{pr}{J}")
                    ctx_of_unit[(pr, J)] = (cx0, cx1)
                pT = emit_scores(pr, J, I)
                pending.append((pr, J, I, last, pT))
                if len(pending) > SKEW:
                    apr, aJ, aI, alast, apT = pending.pop(0)
                    if aI == 0 and aJ >= 1:
                        # a unit's first AV (start=True) must wait for the
                        # previous unit's ctx banks to drain through the DVE
                        # queue — slot useful PE work in front of it
                        forced = 0
                        while (
                            forced < 2
                            and fi < len(fillers)
                            and fillers[fi][0] <= gi
                        ):
                            budget = max(budget - fillers[fi][1], 0.0)
                            fillers[fi][2]()
                            fi += 1
                            forced += 1
                    cx0, cx1 = ctx_of_unit[(apr, aJ)]
                    emit_av(apr, aJ, aI, apT, cx0, cx1)
                    if alast:
                        emit_normalize(apr, aJ, cx0, cx1)
                        del ctx_of_unit[(apr, aJ)]
                # ~131k filler cycles over 80 groups
                budget = min(budget + 1650.0, 8192.0)
                while fi < len(fillers) and fillers[fi][0] <= gi and budget >= fillers[fi][1]:
                    budget -= fillers[fi][1]
                    fillers[fi][2]()
                    fi += 1
            while pending:
                apr, aJ, aI, alast, apT = pending.pop(0)
                cx0, cx1 = ctx_of_unit[(apr, aJ)]
                emit_av(apr, aJ, aI, apT, cx0, cx1)
                if alast:
                    emit_normalize(apr, aJ, cx0, cx1)
                    del ctx_of_unit[(apr, aJ)]
            while fi < len(fillers):
                fillers[fi][2]()
                fi += 1
            for tb in range(NTB - NKB, NTB):
                for ec in range(NEO):
                    emit_out(tb, ec)

    nc.compile()
    return nc


def get_nc():
    global _NC_CACHE
    if _NC_CACHE is None:
        _NC_CACHE = _build_nc()
    return _NC_CACHE


def make_in_maps(x, Wq, Wk, Wv, Wo):
    x = np.asarray(x, dtype=np.float32)
    Wq = np.asarray(Wq, dtype=np.float32)
    Wk = np.asarray(Wk, dtype=np.float32)
    Wv = np.asarray(Wv, dtype=np.float32)
    Wo = np.asarray(Wo, dtype=np.float32)
    b16 = ml_dtypes.bfloat16
    in_maps = []
    for cid in range(N_CORES):
        b, g = divmod(cid, TP)
        sl = slice(DL * g, DL * (g + 1))
        in_maps.append(
            {
                "xT": np.ascontiguousarray(x[b].T).astype(b16),
                "wqT": np.ascontiguousarray(Wq[sl].T).astype(b16),
                "wkT": np.ascontiguousarray(Wk[sl].T).astype(b16),
                "wvT": np.ascontiguousarray(Wv[sl].T).astype(b16),
                "woT": np.ascontiguousarray(Wo[:, sl].T).astype(b16),
            }
        )
    return in_maps


def _combine(results, bo):
    bo = np.asarray(bo, dtype=np.float32)
    y = np.zeros((B, S, E), dtype=np.float32)
    for c in range(N_CORES):
        y[c // TP] += np.asarray(results[c]["out"], dtype=np.float32)
    y += bo
    return y


def kernel(x, Wq, Wk, Wv, Wo, bo):
    nc = get_nc()
    in_maps = make_in_maps(x, Wq, Wk, Wv, Wo)
    res = run_bass_kernel_spmd(nc, in_maps, list(range(N_CORES)))
    return _combine(res.results, bo)


def kernel_traced(x, Wq, Wk, Wv, Wo, bo, trace_cores=None):
    """Like kernel() but with NTFF tracing; returns (output, BassKernelResults)."""
    nc = get_nc()
    in_maps = make_in_maps(x, Wq, Wk, Wv, Wo)
    res = run_bass_kernel_spmd(
        nc, in_maps, list(range(N_CORES)), trace=True, trace_cores=trace_cores
    )
    return _combine(res.results, bo), res


# revision 41
# speedup vs baseline: 1.0385x; 1.0385x over previous
"""Trainium2 Bass kernel for nn_MultiHeadAttention_55894704390646.

Multi-head causal attention, B=2, S=2048, E=1024, H=16 heads, D=64.
Sharding: data-parallel over batch (2 groups) x tensor-parallel over heads
(4 heads per core). Each core computes a partial output-projection result
(row-split Wo); the host sums the 4 partials per batch and adds the bias.

v3 design (all-bf16):
  - every matmul operand is bf16 (PSUM accumulation stays fp32): same
    1 cycle/row as fp32r but half the DMA traffic and SBUF bandwidth, and
    DVE 2x modes for the mask multiplies. End-to-end error ~4e-3 vs the
    2e-2 gate.
  - HW constraint discovered on the way: a matmul's PSUM output must
    start on a 2KB bank boundary (CoreSim accepts unaligned outputs;
    hardware dies). So scores/ctx keep one PSUM bank per head.
  - attention at 512-query chunks, scores transposed [keys, queries];
    exp skips the fully-masked strip of diagonal tiles (partial-width
    ACT + gpsimd memset of the zero strip), recovering most of the
    fine-causal ACT savings; softmax denominator comes from a ones
    column appended to v (M=65 AV), no max subtraction needed.
  - emission interleaves QKV-projection and output-projection matmul
    groups as filler between attention tile-groups (skew-1 score
    prefetch), so the in-order PE queue stays fed and the PE p-state
    clock stays high. Startup DMA is fine-grained (per-ko weight tiles
    interleaved with x chunk 0) so the first projection group starts
    after ~200KB instead of ~4MB.
  - ACT (scalar engine) runs only EXP + the tiny denominator-row
    copies; PSUM evacuations run on the vector engine, reciprocal
    broadcasts on gpsimd.
"""

import os
import sys

if "/opt/trn_rl_repo" not in sys.path:
    sys.path.insert(0, "/opt/trn_rl_repo")

import numpy as np
import ml_dtypes

import concourse.bass as bass
from concourse import bacc
import concourse.mybir as mybir
import concourse.tile as tile
from concourse.bass_utils import run_bass_kernel_spmd

B, S, E, H, D = 2, 2048, 1024, 16, 64
N_CORES = 8
DP = 2                 # batch groups
TP = 4                 # cores per batch group
HL = H // TP           # local heads per core = 4
DL = HL * D            # local head dims = 256
P = 128
NKO = E // P           # contraction blocks over E = 8
CH = 512               # token chunk (projections and attention)
NCH = S // CH          # chunks = 4
NTB = S // P           # 128-token blocks = 16
NPAIR = HL // 2        # head pairs = 2
NEO = E // CH          # output feature chunks of 512 = 2
NKB = CH // P          # k-blocks per chunk = 4
SKEW = 1               # score-group prefetch depth

f32 = mybir.dt.float32
bf16 = mybir.dt.bfloat16
EXP = mybir.ActivationFunctionType.Exp

_NC_CACHE = None


def _build_nc():
    nc = bacc.Bacc("TRN2", target_bir_lowering=False, debug=False)

    xT = nc.dram_tensor("xT", (E, S), bf16, kind="ExternalInput")
    wqT = nc.dram_tensor("wqT", (E, DL), bf16, kind="ExternalInput")
    wkT = nc.dram_tensor("wkT", (E, DL), bf16, kind="ExternalInput")
    wvT = nc.dram_tensor("wvT", (E, DL), bf16, kind="ExternalInput")
    woT = nc.dram_tensor("woT", (DL, E), bf16, kind="ExternalInput")
    out = nc.dram_tensor("out", (S, E), bf16, kind="ExternalOutput")

    with tile.TileContext(nc) as tc:
        with (
            nc.allow_low_precision(reason="bf16 matmuls; validated 4e-3 rel err"),
            tc.tile_pool(name="big", bufs=1) as big,
            tc.tile_pool(name="pt", bufs=4) as ptp,
            tc.tile_pool(name="work", bufs=3) as work,
            tc.tile_pool(name="osb", bufs=3) as osb,
            tc.tile_pool(name="ps", bufs=2, space="PSUM") as ps,
            tc.tile_pool(name="ps_s0", bufs=2, space="PSUM") as ps_s0,
            tc.tile_pool(name="ps_s1", bufs=2, space="PSUM") as ps_s1,
            tc.tile_pool(name="ps_c0", bufs=1, space="PSUM") as ps_c0,
            tc.tile_pool(name="ps_c1", bufs=1, space="PSUM") as ps_c1,
        ):
            # ---------------- DMA loads (fine-grained, startup-ordered) ----
            wq_sb = [None] * NKO
            wk_sb = [None] * NKO
            wv_sb = [None] * NKO
            x_sb = [[None] * NCH for _ in range(NKO)]
            for ko in range(NKO):
                wq_sb[ko] = big.tile([P, DL], bf16, tag=f"wq{ko}", name=f"wq{ko}")
                nc.sync.dma_start(wq_sb[ko][:], wqT[ko * P : (ko + 1) * P, :])
                x_sb[ko][0] = big.tile([P, CH], bf16, tag=f"x{ko}_0", name=f"x{ko}_0")
                nc.sync.dma_start(x_sb[ko][0][:], xT[ko * P : (ko + 1) * P, 0:CH])
            for ko in range(NKO):
                wk_sb[ko] = big.tile([P, DL], bf16, tag=f"wk{ko}", name=f"wk{ko}")
                nc.sync.dma_start(wk_sb[ko][:], wkT[ko * P : (ko + 1) * P, :])
            for ko in range(NKO):
                wv_sb[ko] = big.tile([P, DL], bf16, tag=f"wv{ko}", name=f"wv{ko}")
                nc.sync.dma_start(wv_sb[ko][:], wvT[ko * P : (ko + 1) * P, :])
            for c in range(1, NCH):
                for ko in range(NKO):
                    x_sb[ko][c] = big.tile(
                        [P, CH], bf16, tag=f"x{ko}_{c}", name=f"x{ko}_{c}"
                    )
                    nc.sync.dma_start(
                        x_sb[ko][c][:], xT[ko * P : (ko + 1) * P, c * CH : (c + 1) * CH]
                    )
                if c == 1:
                    wo_sb = big.tile([P, NPAIR, E], bf16, tag="wo")
                    nc.sync.dma_start(
                        wo_sb[:], woT[:].rearrange("(pr p) e -> p pr e", p=P)
                    )

            # causal masks for the 4 diagonal k-blocks of a query chunk:
            # masks[i][k, q] = 1 if k + 128*i <= q else 0  (full width, so
            # the mask multiply also zeroes the stale strip that the
            # partial-width exp leaves below q=128*i)
            masks = []
            for i in range(NKB):
                m = big.tile([P, CH], bf16, tag=f"mask{i}", name=f"mask{i}")
                nc.gpsimd.memset(m[:], 1.0)
                nc.gpsimd.affine_select(
                    out=m[:],
                    in_=m[:],
                    compare_op=mybir.AluOpType.is_ge,
                    fill=0.0,
                    base=-P * i,
                    pattern=[[1, CH]],
                    channel_multiplier=-1,
                )
                masks.append(m)

            ones_stage = big.tile([P, HL], bf16, tag="ones_stage")
            nc.gpsimd.memset(ones_stage[:], 1.0)

            # zero the pT pool buffers once: the diagonal-tile mask multiply
            # reads the strip the partial-width exp skips, which on the first
            # round would otherwise be uninitialized SBUF (NaN * 0 = NaN)
            for i in range(4):
                z = ptp.tile([P, 2, CH], bf16, tag="pT", name=f"pTinit{i}")
                nc.gpsimd.memset(z[:], 0.0)

            # persistent activations
            qT = [[None] * NCH for _ in range(NPAIR)]
            kT = [[None] * NCH for _ in range(NPAIR)]
            for pr in range(NPAIR):
                for c in range(NCH):
                    qT[pr][c] = big.tile([P, CH], bf16, tag=f"qT{pr}{c}", name=f"qT{pr}{c}")
                    kT[pr][c] = big.tile([P, CH], bf16, tag=f"kT{pr}{c}", name=f"kT{pr}{c}")
            v_tb = []
            for tb in range(NTB):
                vt = big.tile([P, HL, D + 2], bf16, tag=f"v{tb}", name=f"v{tb}")
                nc.vector.tensor_copy(vt[:, :, D], ones_stage[:, :])
                v_tb.append(vt)
            ctx_t = []
            for c in range(NCH):
                ctx_t.append(
                    big.tile([P, NPAIR, CH], bf16, tag=f"ctx{c}", name=f"ctx{c}")
                )

            # ---------------- emission helpers ----------------------------
            def emit_qk_group(c, pr, is_k):
                w_sb, dst = (wk_sb, kT) if is_k else (wq_sb, qT)
                pp = ps.tile([P, CH], f32, tag="mm", name=f"pp{c}{pr}{is_k}")
                for ko in range(NKO):
                    nc.tensor.matmul(
                        pp[:],
                        w_sb[ko][:, pr * P : (pr + 1) * P],
                        x_sb[ko][c][:],
                        start=(ko == 0),
                        stop=(ko == NKO - 1),
                    )
                nc.vector.tensor_copy(dst[pr][c][:], pp[:])

            def emit_v_tb(tb):
                pv_full = ps.tile([P, CH], f32, tag="mm", name=f"pv{tb}")
                pv = pv_full[:, 0:DL]
                c, j = tb // NKB, tb % NKB
                for ko in range(NKO):
                    nc.tensor.matmul(
                        pv[:],
                        x_sb[ko][c][:, j * P : (j + 1) * P],
                        wv_sb[ko][:],
                        start=(ko == 0),
                        stop=(ko == NKO - 1),
                    )
                nc.vector.tensor_copy(
                    v_tb[tb][:, :, 0:D], pv[:].rearrange("p (h d) -> p h d", h=HL)
                )

            def emit_scores(pr, J, I):
                """Scores + exp (+diag mask) for k-block I vs query chunk J.
                Per-head PSUM banks (matmul outs must be 2KB-aligned).
                Returns the bf16 pT tile [128, 2, 512]."""
                kc, ks = I // NKB, (I % NKB) * P
                di = I - NKB * J  # >=0 on the diagonal group of blocks
                lo = max(0, di) * P  # queries below lo are fully masked
                s0 = ps_s0.tile([P, CH], f32, tag="s0", name=f"s0_{pr}{J}{I}")
                s1 = ps_s1.tile([P, CH], f32, tag="s1", name=f"s1_{pr}{J}{I}")
                pT = ptp.tile([P, 2, CH], bf16, tag="pT", name=f"pT{pr}{J}{I}")
                for h, sh in ((0, s0), (1, s1)):
                    nc.tensor.matmul(
                        sh[:],
                        kT[pr][kc][h * D : (h + 1) * D, ks : ks + P],
                        qT[pr][J][h * D : (h + 1) * D, :],
                        start=True,
                        stop=True,
                    )
                for h, sh in ((0, s0), (1, s1)):
                    nc.scalar.activation(
                        pT[:, h, lo:CH], sh[:, lo:CH], EXP, scale=0.125
                    )
                    if di >= 0:
                        nc.vector.tensor_tensor(
                            pT[:, h, :],
                            pT[:, h, :],
                            masks[di][:, :],
                            mybir.AluOpType.mult,
                        )
                return pT

            def emit_av(pr, J, I, pT, cx0, cx1):
                nI = NKB * (J + 1)
                last = I == nI - 1
                for h, cx in ((0, cx0), (1, cx1)):
                    nc.tensor.matmul(
                        cx[:], v_tb[I][:, 2 * pr + h, 0 : D + 1], pT[:, h, :],
                        start=(I == 0), stop=last,
                    )

            def emit_normalize(pr, J, cx0, cx1):
                """ctx_t[J][0:64 | 64:128, pr, :] = ctx/den for the pair.
                Each head's full 65 PSUM rows (ctx + den) are evacuated by
                one DVE copy, so the banks free after two DVE ops — the
                recip chain then runs entirely off PSUM. The reciprocal is
                computed in place at partition 64 (no cross-partition copy)
                and broadcast from there. h1 goes first: its path carries
                the extra SBUF-shift DMA latency (split across 4 queues)."""
                cu = work.tile([D, 2, CH], f32, tag="cu", name=f"cu{pr}{J}")
                nc.vector.tensor_copy(cu[:, 0, :], cx0[0:D, :])
                nc.vector.tensor_copy(cu[:, 1, :], cx1[0:D, :])
                dn = work.tile([1, 2, CH], f32, tag="dn", name=f"dn{pr}{J}")
                nc.scalar.copy(dn[:, 0, :], cx0[D : D + 1, :])
                nc.scalar.copy(dn[:, 1, :], cx1[D : D + 1, :])
                recip = work.tile([1, 2, CH], f32, tag="rc", name=f"rc{pr}{J}")
                nc.vector.reciprocal_approx_fast(recip[:], dn[:])
                dnb = work.tile([D, 2, CH], f32, tag="dnb", name=f"dnb{pr}{J}")
                nc.gpsimd.partition_broadcast(dnb[:, 0, :], recip[:, 0, :])
                nc.gpsimd.partition_broadcast(dnb[:, 1, :], recip[:, 1, :])
                # h1 first: its path has the extra SBUF-shift DMA latency
                tmp = work.tile([D, CH], bf16, tag="tmp", name=f"tmp{pr}{J}")
                nc.vector.tensor_tensor(
                    tmp[:], cu[:, 1, :], dnb[:, 1, :], mybir.AluOpType.mult
                )
                if J == NCH - 1:
                    # tail: spread the shift across queues so the final
                    # out-projections aren't gated on one 64KB transfer
                    for j in range(NKB):
                        nc.sync.dma_start(
                            ctx_t[J][D:P, pr, j * P : (j + 1) * P],
                            tmp[:, j * P : (j + 1) * P],
                        )
                else:
                    nc.sync.dma_start(ctx_t[J][D:P, pr, :], tmp[:])
                nc.vector.tensor_tensor(
                    ctx_t[J][0:D, pr, :], cu[:, 0, :], dnb[:, 0, :],
                    mybir.AluOpType.mult,
                )

            def emit_out(tb, ec):
                J, j = tb // NKB, tb % NKB
                o_ps = ps.tile([P, CH], f32, tag="mm", name=f"o{tb}{ec}")
                for pr in range(NPAIR):
                    nc.tensor.matmul(
                        o_ps[:],
                        ctx_t[J][:, pr, j * P : (j + 1) * P],
                        wo_sb[:, pr, ec * CH : (ec + 1) * CH],
                        start=(pr == 0),
                        stop=(pr == NPAIR - 1),
                    )
                o_sb = osb.tile([P, CH], bf16, tag="o_sb", name=f"osb{tb}{ec}")
                nc.vector.tensor_copy(o_sb[:], o_ps[:])
                if tb >= NTB - NKB:
                    # tail: halve the last store latency via two queues
                    half = CH // 2
                    for j in range(2):
                        nc.sync.dma_start(
                            out[
                                tb * P : (tb + 1) * P,
                                ec * CH + j * half : ec * CH + (j + 1) * half,
                            ],
                            o_sb[:, j * half : (j + 1) * half],
                        )
                else:
                    nc.sync.dma_start(
                        out[tb * P : (tb + 1) * P, ec * CH : (ec + 1) * CH], o_sb[:]
                    )

            # ---------------- schedule -------------------------------------
            # q-groups first: the wk DMAs land after wq, so the k-groups
            # get the extra slack of two full projection groups
            for pr in range(NPAIR):
                emit_qk_group(0, pr, False)
            for pr in range(NPAIR):
                emit_qk_group(0, pr, True)
            for tb in range(NKB):
                emit_v_tb(tb)

            units = [(pr, J) for J in range(NCH) for pr in range(NPAIR)]
            glist = []
            for pr, J in units:
                for I in range(NKB * (J + 1)):
                    glist.append((pr, J, I, I == NKB * (J + 1) - 1))

            def win_start(Jw):
                g = 0
                for pr, J in units:
                    if J == Jw:
                        return g
                    g += NKB * (J + 1)
                return len(glist)

            gidx = 0
            unit_end = {}
            for pr, J in units:
                gidx += NKB * (J + 1)
                unit_end[(pr, J)] = gidx

            fillers = []
            for c in range(1, NCH):
                avail = win_start(c - 1)
                for pr in range(NPAIR):
                    fillers.append((avail, 4096, lambda c=c, pr=pr: emit_qk_group(c, pr, False)))
                    fillers.append((avail, 4096, lambda c=c, pr=pr: emit_qk_group(c, pr, True)))
                for tb in range(NKB * c, NKB * c + NKB):
                    fillers.append((avail, 2048, lambda tb=tb: emit_v_tb(tb)))
            for J in range(NCH - 1):  # last chunk's outs go in the tail
                avail = unit_end[(1, J)] + SKEW
                for tb in range(NKB * J, NKB * (J + 1)):
                    for ec in range(NEO):
                        fillers.append((avail, 1024, lambda tb=tb, ec=ec: emit_out(tb, ec)))
            fillers.sort(key=lambda f: f[0])

            ctx_of_unit = {}
            fi = 0
            budget = 0.0
            pending = []
            for gi, (pr, J, I, last) in enumerate(glist):
                if I == 0:
                    cx0 = ps_c0.tile([D + 1, CH], f32, tag="c0", name=f"cx0_{pr}{J}")
                    cx1 = ps_c1.tile([D + 1, CH], f32, tag="c1", name=f"cx1_{pr}{J}")
                    ctx_of_unit[(pr, J)] = (cx0, cx1)
                pT = emit_scores(pr, J, I)
                pending.append((pr, J, I, last, pT))
                if len(pending) > SKEW:
                    apr, aJ, aI, alast, apT = pending.pop(0)
                    cx0, cx1 = ctx_of_unit[(apr, aJ)]
                    emit_av(apr, aJ, aI, apT, cx0, cx1)
                    if alast:
                        emit_normalize(apr, aJ, cx0, cx1)
                        del ctx_of_unit[(apr, aJ)]
                # ~131k filler cycles over 80 groups
                budget = min(budget + 1650.0, 8192.0)
                while fi < len(fillers) and fillers[fi][0] <= gi and budget >= fillers[fi][1]:
                    budget -= fillers[fi][1]
                    fillers[fi][2]()
                    fi += 1
            while pending:
                apr, aJ, aI, alast, apT = pending.pop(0)
                cx0, cx1 = ctx_of_unit[(apr, aJ)]
                emit_av(apr, aJ, aI, apT, cx0, cx1)
                if alast:
                    emit_normalize(apr, aJ, cx0, cx1)
                    del ctx_of_unit[(apr, aJ)]
            while fi < len(fillers):
                fillers[fi][2]()
                fi += 1
            for tb in range(NTB - NKB, NTB):
                for ec in range(NEO):
                    emit_out(tb, ec)

    nc.compile()
    return nc


def get_nc():
    global _NC_CACHE
    if _NC_CACHE is None:
        _NC_CACHE = _build_nc()
    return _NC_CACHE


def make_in_maps(x, Wq, Wk, Wv, Wo):
    x = np.asarray(x, dtype=np.float32)
    Wq = np.asarray(Wq, dtype=np.float32)
    Wk = np.asarray(Wk, dtype=np.float32)
    Wv = np.asarray(Wv, dtype=np.float32)
    Wo = np.asarray(Wo, dtype=np.float32)
    b16 = ml_dtypes.bfloat16
    in_maps = []
    for cid in range(N_CORES):
        b, g = divmod(cid, TP)
        sl = slice(DL * g, DL * (g + 1))
        in_maps.append(
            {
                "xT": np.ascontiguousarray(x[b].T).astype(b16),
                "wqT": np.ascontiguousarray(Wq[sl].T).astype(b16),
                "wkT": np.ascontiguousarray(Wk[sl].T).astype(b16),
                "wvT": np.ascontiguousarray(Wv[sl].T).astype(b16),
                "woT": np.ascontiguousarray(Wo[:, sl].T).astype(b16),
            }
        )
    return in_maps


def _combine(results, bo):
    bo = np.asarray(bo, dtype=np.float32)
    y = np.zeros((B, S, E), dtype=np.float32)
    for c in range(N_CORES):
        y[c // TP] += np.asarray(results[c]["out"], dtype=np.float32)
    y += bo
    return y


def kernel(x, Wq, Wk, Wv, Wo, bo):
    nc = get_nc()
    in_maps = make_in_maps(x, Wq, Wk, Wv, Wo)
    res = run_bass_kernel_spmd(nc, in_maps, list(range(N_CORES)))
    return _combine(res.results, bo)


def kernel_traced(x, Wq, Wk, Wv, Wo, bo, trace_cores=None):
    """Like kernel() but with NTFF tracing; returns (output, BassKernelResults)."""
    nc = get_nc()
    in_maps = make_in_maps(x, Wq, Wk, Wv, Wo)
    res = run_bass_kernel_spmd(
        nc, in_maps, list(range(N_CORES)), trace=True, trace_cores=trace_cores
    )
    return _combine(res.results, bo), res


# revision 44
# speedup vs baseline: 1.0678x; 1.0282x over previous
"""Trainium2 Bass kernel for nn_MultiHeadAttention_55894704390646.

Multi-head causal attention, B=2, S=2048, E=1024, H=16 heads, D=64.
Sharding: data-parallel over batch (2 groups) x tensor-parallel over heads
(4 heads per core). Each core computes a partial output-projection result
(row-split Wo); the host sums the 4 partials per batch and adds the bias.

v3 design (all-bf16):
  - every matmul operand is bf16 (PSUM accumulation stays fp32): same
    1 cycle/row as fp32r but half the DMA traffic and SBUF bandwidth, and
    DVE 2x modes for the mask multiplies. End-to-end error ~4e-3 vs the
    2e-2 gate.
  - HW constraint discovered on the way: a matmul's PSUM output must
    start on a 2KB bank boundary (CoreSim accepts unaligned outputs;
    hardware dies). So scores/ctx keep one PSUM bank per head.
  - attention at 512-query chunks, scores transposed [keys, queries];
    exp skips the fully-masked strip of diagonal tiles (partial-width
    ACT), and the full-width diagonal mask multiply zeroes the skipped
    strip as a side effect — no per-tile memsets, which keeps gpsimd
    free for the reciprocal broadcasts (mixing memsets in caused
    library-reload thrash and a 6us all-engine stall every chunk).
    The softmax denominator comes from a ones column appended to v
    (M=65 AV); no max subtraction needed (|s|/8 bounded ~+-6).
  - emission interleaves QKV-projection and output-projection matmul
    groups as filler between attention tile-groups (skew-1 score
    prefetch), so the in-order PE queue stays fed and the PE p-state
    clock stays high. Startup DMA is fine-grained (per-ko weight tiles
    interleaved with x chunk 0) so the first projection group starts
    after ~200KB instead of ~4MB.
  - ACT (scalar engine) runs only EXP + the tiny denominator-row
    copies; PSUM evacuations run on the vector engine, reciprocal
    broadcasts on gpsimd.
"""

import os
import sys

if "/opt/trn_rl_repo" not in sys.path:
    sys.path.insert(0, "/opt/trn_rl_repo")

import numpy as np
import ml_dtypes

import concourse.bass as bass
from concourse import bacc
import concourse.mybir as mybir
import concourse.tile as tile
from concourse.bass_utils import run_bass_kernel_spmd

B, S, E, H, D = 2, 2048, 1024, 16, 64
N_CORES = 8
DP = 2                 # batch groups
TP = 4                 # cores per batch group
HL = H // TP           # local heads per core = 4
DL = HL * D            # local head dims = 256
P = 128
NKO = E // P           # contraction blocks over E = 8
CH = 512               # token chunk (projections and attention)
NCH = S // CH          # chunks = 4
NTB = S // P           # 128-token blocks = 16
NPAIR = HL // 2        # head pairs = 2
NEO = E // CH          # output feature chunks of 512 = 2
NKB = CH // P          # k-blocks per chunk = 4
SKEW = 1               # score-group prefetch depth

f32 = mybir.dt.float32
bf16 = mybir.dt.bfloat16
EXP = mybir.ActivationFunctionType.Exp

_NC_CACHE = None


def _build_nc():
    nc = bacc.Bacc("TRN2", target_bir_lowering=False, debug=False)

    xT = nc.dram_tensor("xT", (E, S), bf16, kind="ExternalInput")
    wqT = nc.dram_tensor("wqT", (E, DL), bf16, kind="ExternalInput")
    wkT = nc.dram_tensor("wkT", (E, DL), bf16, kind="ExternalInput")
    wvT = nc.dram_tensor("wvT", (E, DL), bf16, kind="ExternalInput")
    woT = nc.dram_tensor("woT", (DL, E), bf16, kind="ExternalInput")
    out = nc.dram_tensor("out", (S, E), bf16, kind="ExternalOutput")

    with tile.TileContext(nc) as tc:
        with (
            nc.allow_low_precision(reason="bf16 matmuls; validated 4e-3 rel err"),
            tc.tile_pool(name="big", bufs=1) as big,
            tc.tile_pool(name="pt", bufs=4) as ptp,
            tc.tile_pool(name="work", bufs=3) as work,
            tc.tile_pool(name="osb", bufs=3) as osb,
            tc.tile_pool(name="ps", bufs=2, space="PSUM") as ps,
            tc.tile_pool(name="ps_s0", bufs=2, space="PSUM") as ps_s0,
            tc.tile_pool(name="ps_s1", bufs=2, space="PSUM") as ps_s1,
            tc.tile_pool(name="ps_c0", bufs=1, space="PSUM") as ps_c0,
            tc.tile_pool(name="ps_c1", bufs=1, space="PSUM") as ps_c1,
        ):
            # ---------------- DMA loads (fine-grained, startup-ordered) ----
            wq_sb = [None] * NKO
            wk_sb = [None] * NKO
            wv_sb = [None] * NKO
            x_sb = [[None] * NCH for _ in range(NKO)]
            for ko in range(NKO):
                wq_sb[ko] = big.tile([P, DL], bf16, tag=f"wq{ko}", name=f"wq{ko}")
                nc.sync.dma_start(wq_sb[ko][:], wqT[ko * P : (ko + 1) * P, :])
                x_sb[ko][0] = big.tile([P, CH], bf16, tag=f"x{ko}_0", name=f"x{ko}_0")
                nc.sync.dma_start(x_sb[ko][0][:], xT[ko * P : (ko + 1) * P, 0:CH])
            for ko in range(NKO):
                wk_sb[ko] = big.tile([P, DL], bf16, tag=f"wk{ko}", name=f"wk{ko}")
                nc.sync.dma_start(wk_sb[ko][:], wkT[ko * P : (ko + 1) * P, :])
            for ko in range(NKO):
                wv_sb[ko] = big.tile([P, DL], bf16, tag=f"wv{ko}", name=f"wv{ko}")
                nc.sync.dma_start(wv_sb[ko][:], wvT[ko * P : (ko + 1) * P, :])
            for c in range(1, NCH):
                for ko in range(NKO):
                    x_sb[ko][c] = big.tile(
                        [P, CH], bf16, tag=f"x{ko}_{c}", name=f"x{ko}_{c}"
                    )
                    nc.sync.dma_start(
                        x_sb[ko][c][:], xT[ko * P : (ko + 1) * P, c * CH : (c + 1) * CH]
                    )
                if c == 1:
                    wo_sb = big.tile([P, NPAIR, E], bf16, tag="wo")
                    nc.sync.dma_start(
                        wo_sb[:], woT[:].rearrange("(pr p) e -> p pr e", p=P)
                    )

            # causal masks for the 4 diagonal k-blocks of a query chunk:
            # masks[i][k, q] = 1 if k + 128*i <= q else 0  (full width, so
            # the mask multiply also zeroes the stale strip that the
            # partial-width exp leaves below q=128*i)
            masks = []
            for i in range(NKB):
                m = big.tile([P, CH], bf16, tag=f"mask{i}", name=f"mask{i}")
                nc.gpsimd.memset(m[:], 1.0)
                nc.gpsimd.affine_select(
                    out=m[:],
                    in_=m[:],
                    compare_op=mybir.AluOpType.is_ge,
                    fill=0.0,
                    base=-P * i,
                    pattern=[[1, CH]],
                    channel_multiplier=-1,
                )
                masks.append(m)

            ones_stage = big.tile([P, HL], bf16, tag="ones_stage")
            nc.gpsimd.memset(ones_stage[:], 1.0)

            # zero the pT pool buffers once: the diagonal-tile mask multiply
            # reads the strip the partial-width exp skips, which on the first
            # round would otherwise be uninitialized SBUF (NaN * 0 = NaN)
            for i in range(4):
                z = ptp.tile([P, 2, CH], bf16, tag="pT", name=f"pTinit{i}")
                nc.gpsimd.memset(z[:], 0.0)

            # persistent activations
            qT = [[None] * NCH for _ in range(NPAIR)]
            kT = [[None] * NCH for _ in range(NPAIR)]
            for pr in range(NPAIR):
                for c in range(NCH):
                    qT[pr][c] = big.tile([P, CH], bf16, tag=f"qT{pr}{c}", name=f"qT{pr}{c}")
                    kT[pr][c] = big.tile([P, CH], bf16, tag=f"kT{pr}{c}", name=f"kT{pr}{c}")
            v_tb = []
            for tb in range(NTB):
                vt = big.tile([P, HL, D + 2], bf16, tag=f"v{tb}", name=f"v{tb}")
                nc.vector.tensor_copy(vt[:, :, D], ones_stage[:, :])
                v_tb.append(vt)
            ctx_t = []
            for c in range(NCH):
                ctx_t.append(
                    big.tile([P, NPAIR, CH], bf16, tag=f"ctx{c}", name=f"ctx{c}")
                )

            # ---------------- emission helpers ----------------------------
            def emit_qk_group(c, pr, is_k):
                w_sb, dst = (wk_sb, kT) if is_k else (wq_sb, qT)
                pp = ps.tile([P, CH], f32, tag="mm", name=f"pp{c}{pr}{is_k}")
                for ko in range(NKO):
                    nc.tensor.matmul(
                        pp[:],
                        w_sb[ko][:, pr * P : (pr + 1) * P],
                        x_sb[ko][c][:],
                        start=(ko == 0),
                        stop=(ko == NKO - 1),
                    )
                nc.vector.tensor_copy(dst[pr][c][:], pp[:])

            def emit_v_tb(tb):
                pv_full = ps.tile([P, CH], f32, tag="mm", name=f"pv{tb}")
                pv = pv_full[:, 0:DL]
                c, j = tb // NKB, tb % NKB
                for ko in range(NKO):
                    nc.tensor.matmul(
                        pv[:],
                        x_sb[ko][c][:, j * P : (j + 1) * P],
                        wv_sb[ko][:],
                        start=(ko == 0),
                        stop=(ko == NKO - 1),
                    )
                nc.vector.tensor_copy(
                    v_tb[tb][:, :, 0:D], pv[:].rearrange("p (h d) -> p h d", h=HL)
                )

            def emit_scores(pr, J, I):
                """Scores + exp (+diag mask) for k-block I vs query chunk J.
                Per-head PSUM banks (matmul outs must be 2KB-aligned).
                Returns the bf16 pT tile [128, 2, 512]."""
                kc, ks = I // NKB, (I % NKB) * P
                di = I - NKB * J  # >=0 on the diagonal group of blocks
                lo = max(0, di) * P  # queries below lo are fully masked
                s0 = ps_s0.tile([P, CH], f32, tag="s0", name=f"s0_{pr}{J}{I}")
                s1 = ps_s1.tile([P, CH], f32, tag="s1", name=f"s1_{pr}{J}{I}")
                pT = ptp.tile([P, 2, CH], bf16, tag="pT", name=f"pT{pr}{J}{I}")
                for h, sh in ((0, s0), (1, s1)):
                    nc.tensor.matmul(
                        sh[:],
                        kT[pr][kc][h * D : (h + 1) * D, ks : ks + P],
                        qT[pr][J][h * D : (h + 1) * D, :],
                        start=True,
                        stop=True,
                    )
                for h, sh in ((0, s0), (1, s1)):
                    nc.scalar.activation(
                        pT[:, h, lo:CH], sh[:, lo:CH], EXP, scale=0.125
                    )
                    if di >= 0:
                        nc.vector.tensor_tensor(
                            pT[:, h, :],
                            pT[:, h, :],
                            masks[di][:, :],
                            mybir.AluOpType.mult,
                        )
                return pT

            def emit_av(pr, J, I, pT, cx0, cx1):
                nI = NKB * (J + 1)
                last = I == nI - 1
                for h, cx in ((0, cx0), (1, cx1)):
                    nc.tensor.matmul(
                        cx[:], v_tb[I][:, 2 * pr + h, 0 : D + 1], pT[:, h, :],
                        start=(I == 0), stop=last,
                    )

            def emit_normalize(pr, J, cx0, cx1):
                """ctx_t[J][0:64 | 64:128, pr, :] = ctx/den for the pair.
                Each head's full 65 PSUM rows (ctx + den) are evacuated by
                one DVE copy, so the banks free after two DVE ops — the
                recip chain then runs entirely off PSUM. The reciprocal is
                computed in place at partition 64 (no cross-partition copy)
                and broadcast from there. h1 goes first: its path carries
                the extra SBUF-shift DMA latency (split across 4 queues)."""
                # h1's whole chain runs first (it feeds the SBUF-shift DMA);
                # h0's recip/broadcast overlap behind it on ACT/DVE/gpsimd
                cu = work.tile([D, 2, CH], f32, tag="cu", name=f"cu{pr}{J}")
                dn = work.tile([1, 2, CH], f32, tag="dn", name=f"dn{pr}{J}")
                rc = work.tile([1, 2, CH], f32, tag="rc", name=f"rc{pr}{J}")
                dnb = work.tile([D, 2, CH], f32, tag="dnb", name=f"dnb{pr}{J}")
                tmp = work.tile([D, CH], bf16, tag="tmp", name=f"tmp{pr}{J}")
                nc.scalar.copy(dn[:, 1, :], cx1[D : D + 1, :])
                nc.scalar.copy(dn[:, 0, :], cx0[D : D + 1, :])
                nc.vector.tensor_copy(cu[:, 1, :], cx1[0:D, :])
                nc.vector.reciprocal_approx_fast(rc[:, 1, :], dn[:, 1, :])
                nc.gpsimd.partition_broadcast(dnb[:, 1, :], rc[:, 1, :])
                nc.vector.tensor_copy(cu[:, 0, :], cx0[0:D, :])
                nc.vector.reciprocal_approx_fast(rc[:, 0, :], dn[:, 0, :])
                nc.gpsimd.partition_broadcast(dnb[:, 0, :], rc[:, 0, :])
                nc.vector.tensor_tensor(
                    tmp[:], cu[:, 1, :], dnb[:, 1, :], mybir.AluOpType.mult
                )
                nc.sync.dma_start(ctx_t[J][D:P, pr, :], tmp[:])
                nc.vector.tensor_tensor(
                    ctx_t[J][0:D, pr, :], cu[:, 0, :], dnb[:, 0, :],
                    mybir.AluOpType.mult,
                )

            def emit_out(tb, ec):
                J, j = tb // NKB, tb % NKB
                o_ps = ps.tile([P, CH], f32, tag="mm", name=f"o{tb}{ec}")
                for pr in range(NPAIR):
                    nc.tensor.matmul(
                        o_ps[:],
                        ctx_t[J][:, pr, j * P : (j + 1) * P],
                        wo_sb[:, pr, ec * CH : (ec + 1) * CH],
                        start=(pr == 0),
                        stop=(pr == NPAIR - 1),
                    )
                o_sb = osb.tile([P, CH], bf16, tag="o_sb", name=f"osb{tb}{ec}")
                nc.vector.tensor_copy(o_sb[:], o_ps[:])
                # one dma_start only: every extra descriptor costs ~600ns
                # of serialized DIRECT2D work on the sync engine
                nc.sync.dma_start(
                    out[tb * P : (tb + 1) * P, ec * CH : (ec + 1) * CH], o_sb[:]
                )

            # ---------------- schedule -------------------------------------
            # q-groups first: the wk DMAs land after wq, so the k-groups
            # get the extra slack of two full projection groups
            for pr in range(NPAIR):
                emit_qk_group(0, pr, False)
            for pr in range(NPAIR):
                emit_qk_group(0, pr, True)
            for tb in range(NKB):
                emit_v_tb(tb)

            units = [(pr, J) for J in range(NCH) for pr in range(NPAIR)]
            glist = []
            for pr, J in units:
                for I in range(NKB * (J + 1)):
                    glist.append((pr, J, I, I == NKB * (J + 1) - 1))

            def win_start(Jw):
                g = 0
                for pr, J in units:
                    if J == Jw:
                        return g
                    g += NKB * (J + 1)
                return len(glist)

            gidx = 0
            unit_end = {}
            for pr, J in units:
                gidx += NKB * (J + 1)
                unit_end[(pr, J)] = gidx

            fillers = []
            for c in range(1, NCH):
                avail = win_start(c - 1)
                for pr in range(NPAIR):
                    fillers.append((avail, 4096, lambda c=c, pr=pr: emit_qk_group(c, pr, False)))
                    fillers.append((avail, 4096, lambda c=c, pr=pr: emit_qk_group(c, pr, True)))
                for tb in range(NKB * c, NKB * c + NKB):
                    fillers.append((avail, 2048, lambda tb=tb: emit_v_tb(tb)))
            for J in range(NCH - 1):  # last chunk's outs go in the tail
                avail = unit_end[(1, J)] + SKEW
                for tb in range(NKB * J, NKB * (J + 1)):
                    for ec in range(NEO):
                        fillers.append((avail, 1024, lambda tb=tb, ec=ec: emit_out(tb, ec)))
            fillers.sort(key=lambda f: f[0])

            ctx_of_unit = {}
            fi = 0
            budget = 0.0
            pending = []
            for gi, (pr, J, I, last) in enumerate(glist):
                if I == 0:
                    cx0 = ps_c0.tile([D + 1, CH], f32, tag="c0", name=f"cx0_{pr}{J}")
                    cx1 = ps_c1.tile([D + 1, CH], f32, tag="c1", name=f"cx1_{pr}{J}")
                    ctx_of_unit[(pr, J)] = (cx0, cx1)
                pT = emit_scores(pr, J, I)
                pending.append((pr, J, I, last, pT))
                if len(pending) > SKEW:
                    apr, aJ, aI, alast, apT = pending.pop(0)
                    cx0, cx1 = ctx_of_unit[(apr, aJ)]
                    emit_av(apr, aJ, aI, apT, cx0, cx1)
                    if alast:
                        emit_normalize(apr, aJ, cx0, cx1)
                        del ctx_of_unit[(apr, aJ)]
                # ~131k filler cycles over 80 groups
                budget = min(budget + 1650.0, 8192.0)
                while fi < len(fillers) and fillers[fi][0] <= gi and budget >= fillers[fi][1]:
                    budget -= fillers[fi][1]
                    fillers[fi][2]()
                    fi += 1
            while pending:
                apr, aJ, aI, alast, apT = pending.pop(0)
                cx0, cx1 = ctx_of_unit[(apr, aJ)]
                emit_av(apr, aJ, aI, apT, cx0, cx1)
                if alast:
                    emit_normalize(apr, aJ, cx0, cx1)
                    del ctx_of_unit[(apr, aJ)]
            while fi < len(fillers):
                fillers[fi][2]()
                fi += 1
            for tb in range(NTB - NKB, NTB):
                for ec in range(NEO):
                    emit_out(tb, ec)

    nc.compile()
    return nc


def get_nc():
    global _NC_CACHE
    if _NC_CACHE is None:
        _NC_CACHE = _build_nc()
    return _NC_CACHE


def make_in_maps(x, Wq, Wk, Wv, Wo):
    x = np.asarray(x, dtype=np.float32)
    Wq = np.asarray(Wq, dtype=np.float32)
    Wk = np.asarray(Wk, dtype=np.float32)
    Wv = np.asarray(Wv, dtype=np.float32)
    Wo = np.asarray(Wo, dtype=np.float32)
    b16 = ml_dtypes.bfloat16
    in_maps = []
    for cid in range(N_CORES):
        b, g = divmod(cid, TP)
        sl = slice(DL * g, DL * (g + 1))
        in_maps.append(
            {
                "xT": np.ascontiguousarray(x[b].T).astype(b16),
                "wqT": np.ascontiguousarray(Wq[sl].T).astype(b16),
                "wkT": np.ascontiguousarray(Wk[sl].T).astype(b16),
                "wvT": np.ascontiguousarray(Wv[sl].T).astype(b16),
                "woT": np.ascontiguousarray(Wo[:, sl].T).astype(b16),
            }
        )
    return in_maps


def _combine(results, bo):
    bo = np.asarray(bo, dtype=np.float32)
    y = np.zeros((B, S, E), dtype=np.float32)
    for c in range(N_CORES):
        y[c // TP] += np.asarray(results[c]["out"], dtype=np.float32)
    y += bo
    return y


def kernel(x, Wq, Wk, Wv, Wo, bo):
    nc = get_nc()
    in_maps = make_in_maps(x, Wq, Wk, Wv, Wo)
    res = run_bass_kernel_spmd(nc, in_maps, list(range(N_CORES)))
    return _combine(res.results, bo)


def kernel_traced(x, Wq, Wk, Wv, Wo, bo, trace_cores=None):
    """Like kernel() but with NTFF tracing; returns (output, BassKernelResults)."""
    nc = get_nc()
    in_maps = make_in_maps(x, Wq, Wk, Wv, Wo)
    res = run_bass_kernel_spmd(
        nc, in_maps, list(range(N_CORES)), trace=True, trace_cores=trace_cores
    )
    return _combine(res.results, bo), res


# revision 47
# speedup vs baseline: 1.0861x; 1.0172x over previous
"""Trainium2 Bass kernel for nn_MultiHeadAttention_55894704390646.

Multi-head causal attention, B=2, S=2048, E=1024, H=16 heads, D=64.
Sharding: data-parallel over batch (2 groups) x tensor-parallel over heads
(4 heads per core). Each core computes a partial output-projection result
(row-split Wo); the host sums the 4 partials per batch and adds the bias.

v3 design (all-bf16):
  - every matmul operand is bf16 (PSUM accumulation stays fp32): same
    1 cycle/row as fp32r but half the DMA traffic and SBUF bandwidth, and
    DVE 2x modes for the mask multiplies. End-to-end error ~4e-3 vs the
    2e-2 gate.
  - HW constraint discovered on the way: a matmul's PSUM output must
    start on a 2KB bank boundary (CoreSim accepts unaligned outputs;
    hardware dies). So scores/ctx keep one PSUM bank per head.
  - attention at 512-query chunks, scores transposed [keys, queries];
    exp skips the fully-masked strip of diagonal tiles (partial-width
    ACT), and the full-width diagonal mask multiply zeroes the skipped
    strip as a side effect — no per-tile memsets, which keeps gpsimd
    free for the reciprocal broadcasts (mixing memsets in caused
    library-reload thrash and a 6us all-engine stall every chunk).
    The softmax denominator comes from a ones column appended to v
    (M=65 AV); no max subtraction needed (|s|/8 bounded ~+-6).
  - emission interleaves QKV-projection and output-projection matmul
    groups as filler between attention tile-groups (skew-1 score
    prefetch), so the in-order PE queue stays fed and the PE p-state
    clock stays high. Startup DMA is fine-grained (per-ko weight tiles
    interleaved with x chunk 0) so the first projection group starts
    after ~200KB instead of ~4MB.
  - ACT (scalar engine) runs only EXP + the tiny denominator-row
    copies; PSUM evacuations run on the vector engine, reciprocal
    broadcasts on gpsimd.
"""

import os
import sys

if "/opt/trn_rl_repo" not in sys.path:
    sys.path.insert(0, "/opt/trn_rl_repo")

import numpy as np
import ml_dtypes

import concourse.bass as bass
from concourse import bacc
import concourse.mybir as mybir
import concourse.tile as tile
from concourse.bass_utils import run_bass_kernel_spmd

B, S, E, H, D = 2, 2048, 1024, 16, 64
N_CORES = 8
DP = 2                 # batch groups
TP = 4                 # cores per batch group
HL = H // TP           # local heads per core = 4
DL = HL * D            # local head dims = 256
P = 128
NKO = E // P           # contraction blocks over E = 8
CH = 512               # token chunk (projections and attention)
NCH = S // CH          # chunks = 4
NTB = S // P           # 128-token blocks = 16
NPAIR = HL // 2        # head pairs = 2
NEO = E // CH          # output feature chunks of 512 = 2
NKB = CH // P          # k-blocks per chunk = 4
SKEW = 1               # score-group prefetch depth

f32 = mybir.dt.float32
bf16 = mybir.dt.bfloat16
EXP = mybir.ActivationFunctionType.Exp

_NC_CACHE = None


def _build_nc():
    nc = bacc.Bacc("TRN2", target_bir_lowering=False, debug=False)

    xT = nc.dram_tensor("xT", (E, S), bf16, kind="ExternalInput")
    wqT = nc.dram_tensor("wqT", (E, DL), bf16, kind="ExternalInput")
    wkT = nc.dram_tensor("wkT", (E, DL), bf16, kind="ExternalInput")
    wvT = nc.dram_tensor("wvT", (E, DL), bf16, kind="ExternalInput")
    woT = nc.dram_tensor("woT", (DL, E), bf16, kind="ExternalInput")
    out = nc.dram_tensor("out", (S, E), bf16, kind="ExternalOutput")

    with tile.TileContext(nc) as tc:
        with (
            nc.allow_low_precision(reason="bf16 matmuls; validated 4e-3 rel err"),
            tc.tile_pool(name="big", bufs=1) as big,
            tc.tile_pool(name="pt", bufs=4) as ptp,
            tc.tile_pool(name="work", bufs=3) as work,
            tc.tile_pool(name="osb", bufs=3) as osb,
            tc.tile_pool(name="ps", bufs=2, space="PSUM") as ps,
            tc.tile_pool(name="ps_s0", bufs=2, space="PSUM") as ps_s0,
            tc.tile_pool(name="ps_s1", bufs=2, space="PSUM") as ps_s1,
            tc.tile_pool(name="ps_c0", bufs=1, space="PSUM") as ps_c0,
            tc.tile_pool(name="ps_c1", bufs=1, space="PSUM") as ps_c1,
        ):
            # ---------------- DMA loads (fine-grained, startup-ordered) ----
            wq_sb = [None] * NKO
            wk_sb = [None] * NKO
            wv_sb = [None] * NKO
            x_sb = [[None] * NCH for _ in range(NKO)]
            for ko in range(NKO):
                wq_sb[ko] = big.tile([P, DL], bf16, tag=f"wq{ko}", name=f"wq{ko}")
                nc.sync.dma_start(wq_sb[ko][:], wqT[ko * P : (ko + 1) * P, :])
                x_sb[ko][0] = big.tile([P, CH], bf16, tag=f"x{ko}_0", name=f"x{ko}_0")
                nc.sync.dma_start(x_sb[ko][0][:], xT[ko * P : (ko + 1) * P, 0:CH])
            # non-startup-critical tensors load as single DMAs: every extra
            # dma_start costs ~600ns of serialized DIRECT2D on the sync engine
            wk_m = big.tile([P, NKO, DL], bf16, tag="wk_m")
            nc.sync.dma_start(wk_m[:], wkT[:].rearrange("(ko p) d -> p ko d", p=P))
            wv_m = big.tile([P, NKO, DL], bf16, tag="wv_m")
            nc.sync.dma_start(wv_m[:], wvT[:].rearrange("(ko p) d -> p ko d", p=P))
            x_m = [None] * NCH
            for c in range(1, NCH):
                x_m[c] = big.tile([P, NKO, CH], bf16, tag=f"x_m{c}", name=f"x_m{c}")
                nc.sync.dma_start(
                    x_m[c][:],
                    xT[:, c * CH : (c + 1) * CH].rearrange("(ko p) s -> p ko s", p=P),
                )
                if c == 1:
                    wo_sb = big.tile([P, NPAIR, E], bf16, tag="wo")
                    nc.sync.dma_start(
                        wo_sb[:], woT[:].rearrange("(pr p) e -> p pr e", p=P)
                    )



            # causal masks for the 4 diagonal k-blocks of a query chunk:
            # masks[i][k, q] = 1 if k + 128*i <= q else 0  (full width, so
            # the mask multiply also zeroes the stale strip that the
            # partial-width exp leaves below q=128*i)
            masks = []
            for i in range(NKB):
                m = big.tile([P, CH], bf16, tag=f"mask{i}", name=f"mask{i}")
                nc.gpsimd.memset(m[:], 1.0)
                nc.gpsimd.affine_select(
                    out=m[:],
                    in_=m[:],
                    compare_op=mybir.AluOpType.is_ge,
                    fill=0.0,
                    base=-P * i,
                    pattern=[[1, CH]],
                    channel_multiplier=-1,
                )
                masks.append(m)

            ones_stage = big.tile([P, HL], bf16, tag="ones_stage")
            nc.gpsimd.memset(ones_stage[:], 1.0)

            # zero the pT pool buffers once: the diagonal-tile mask multiply
            # reads the strip the partial-width exp skips, which on the first
            # round would otherwise be uninitialized SBUF (NaN * 0 = NaN)
            for i in range(4):
                z = ptp.tile([P, 2, CH], bf16, tag="pT", name=f"pTinit{i}")
                nc.gpsimd.memset(z[:], 0.0)

            # persistent activations
            qT = [[None] * NCH for _ in range(NPAIR)]
            kT = [[None] * NCH for _ in range(NPAIR)]
            for pr in range(NPAIR):
                for c in range(NCH):
                    qT[pr][c] = big.tile([P, CH], bf16, tag=f"qT{pr}{c}", name=f"qT{pr}{c}")
                    kT[pr][c] = big.tile([P, CH], bf16, tag=f"kT{pr}{c}", name=f"kT{pr}{c}")
            v_tb = []
            for tb in range(NTB):
                vt = big.tile([P, HL, D + 2], bf16, tag=f"v{tb}", name=f"v{tb}")
                nc.vector.tensor_copy(vt[:, :, D], ones_stage[:, :])
                v_tb.append(vt)
            ctx_t = []
            for c in range(NCH):
                ctx_t.append(
                    big.tile([P, NPAIR, CH], bf16, tag=f"ctx{c}", name=f"ctx{c}")
                )

            # ---------------- emission helpers ----------------------------
            def emit_qk_group(c, pr, is_k):
                dst = kT if is_k else qT
                pp = ps.tile([P, CH], f32, tag="mm", name=f"pp{c}{pr}{is_k}")
                for ko in range(NKO):
                    stat = (
                        wk_m[:, ko, pr * P : (pr + 1) * P]
                        if is_k
                        else wq_sb[ko][:, pr * P : (pr + 1) * P]
                    )
                    mov = x_sb[ko][0][:] if c == 0 else x_m[c][:, ko, :]
                    nc.tensor.matmul(
                        pp[:], stat, mov,
                        start=(ko == 0),
                        stop=(ko == NKO - 1),
                    )
                nc.vector.tensor_copy(dst[pr][c][:], pp[:])

            def emit_v_tb(tb):
                pv_full = ps.tile([P, CH], f32, tag="mm", name=f"pv{tb}")
                pv = pv_full[:, 0:DL]
                c, j = tb // NKB, tb % NKB
                for ko in range(NKO):
                    stat = (
                        x_sb[ko][0][:, j * P : (j + 1) * P]
                        if c == 0
                        else x_m[c][:, ko, j * P : (j + 1) * P]
                    )
                    nc.tensor.matmul(
                        pv[:],
                        stat,
                        wv_m[:, ko, :],
                        start=(ko == 0),
                        stop=(ko == NKO - 1),
                    )
                nc.vector.tensor_copy(
                    v_tb[tb][:, :, 0:D], pv[:].rearrange("p (h d) -> p h d", h=HL)
                )

            def emit_scores(pr, J, I):
                """Scores + exp (+diag mask) for k-block I vs query chunk J.
                Per-head PSUM banks (matmul outs must be 2KB-aligned).
                Returns the bf16 pT tile [128, 2, 512]."""
                kc, ks = I // NKB, (I % NKB) * P
                di = I - NKB * J  # >=0 on the diagonal group of blocks
                lo = max(0, di) * P  # queries below lo are fully masked
                s0 = ps_s0.tile([P, CH], f32, tag="s0", name=f"s0_{pr}{J}{I}")
                s1 = ps_s1.tile([P, CH], f32, tag="s1", name=f"s1_{pr}{J}{I}")
                pT = ptp.tile([P, 2, CH], bf16, tag="pT", name=f"pT{pr}{J}{I}")
                for h, sh in ((0, s0), (1, s1)):
                    nc.tensor.matmul(
                        sh[:],
                        kT[pr][kc][h * D : (h + 1) * D, ks : ks + P],
                        qT[pr][J][h * D : (h + 1) * D, :],
                        start=True,
                        stop=True,
                    )
                for h, sh in ((0, s0), (1, s1)):
                    nc.scalar.activation(
                        pT[:, h, lo:CH], sh[:, lo:CH], EXP, scale=0.125
                    )
                    if di >= 0:
                        nc.vector.tensor_tensor(
                            pT[:, h, :],
                            pT[:, h, :],
                            masks[di][:, :],
                            mybir.AluOpType.mult,
                        )
                return pT

            def emit_av(pr, J, I, pT, cx0, cx1):
                nI = NKB * (J + 1)
                last = I == nI - 1
                for h, cx in ((0, cx0), (1, cx1)):
                    nc.tensor.matmul(
                        cx[:], v_tb[I][:, 2 * pr + h, 0 : D + 1], pT[:, h, :],
                        start=(I == 0), stop=last,
                    )

            def emit_normalize(pr, J, cx0, cx1):
                """ctx_t[J][0:64 | 64:128, pr, :] = ctx/den for the pair.
                Each head's full 65 PSUM rows (ctx + den) are evacuated by
                one DVE copy, so the banks free after two DVE ops — the
                recip chain then runs entirely off PSUM. The reciprocal is
                computed in place at partition 64 (no cross-partition copy)
                and broadcast from there. h1 goes first: its path carries
                the extra SBUF-shift DMA latency (split across 4 queues)."""
                # h1's whole chain runs first (it feeds the SBUF-shift DMA);
                # h0's recip/broadcast overlap behind it on ACT/DVE/gpsimd
                cu = work.tile([D, 2, CH], f32, tag="cu", name=f"cu{pr}{J}")
                dn = work.tile([1, 2, CH], f32, tag="dn", name=f"dn{pr}{J}")
                rc = work.tile([1, 2, CH], f32, tag="rc", name=f"rc{pr}{J}")
                dnb = work.tile([D, 2, CH], f32, tag="dnb", name=f"dnb{pr}{J}")
                tmp = work.tile([D, CH], bf16, tag="tmp", name=f"tmp{pr}{J}")
                nc.scalar.copy(dn[:, 1, :], cx1[D : D + 1, :])
                nc.scalar.copy(dn[:, 0, :], cx0[D : D + 1, :])
                nc.vector.tensor_copy(cu[:, 1, :], cx1[0:D, :])
                nc.vector.reciprocal_approx_fast(rc[:, 1, :], dn[:, 1, :])
                nc.gpsimd.partition_broadcast(dnb[:, 1, :], rc[:, 1, :])
                nc.vector.tensor_copy(cu[:, 0, :], cx0[0:D, :])
                nc.vector.reciprocal_approx_fast(rc[:, 0, :], dn[:, 0, :])
                nc.gpsimd.partition_broadcast(dnb[:, 0, :], rc[:, 0, :])
                nc.vector.tensor_tensor(
                    tmp[:], cu[:, 1, :], dnb[:, 1, :], mybir.AluOpType.mult
                )
                nc.sync.dma_start(ctx_t[J][D:P, pr, :], tmp[:])
                nc.vector.tensor_tensor(
                    ctx_t[J][0:D, pr, :], cu[:, 0, :], dnb[:, 0, :],
                    mybir.AluOpType.mult,
                )

            def emit_out(tb, ec):
                J, j = tb // NKB, tb % NKB
                o_ps = ps.tile([P, CH], f32, tag="mm", name=f"o{tb}{ec}")
                for pr in range(NPAIR):
                    nc.tensor.matmul(
                        o_ps[:],
                        ctx_t[J][:, pr, j * P : (j + 1) * P],
                        wo_sb[:, pr, ec * CH : (ec + 1) * CH],
                        start=(pr == 0),
                        stop=(pr == NPAIR - 1),
                    )
                o_sb = osb.tile([P, CH], bf16, tag="o_sb", name=f"osb{tb}{ec}")
                nc.vector.tensor_copy(o_sb[:], o_ps[:])
                # one dma_start only: every extra descriptor costs ~600ns
                # of serialized DIRECT2D work on the sync engine
                nc.sync.dma_start(
                    out[tb * P : (tb + 1) * P, ec * CH : (ec + 1) * CH], o_sb[:]
                )

            # ---------------- schedule -------------------------------------
            # q-groups first: the wk DMAs land after wq, so the k-groups
            # get the extra slack of two full projection groups
            for pr in range(NPAIR):
                emit_qk_group(0, pr, False)
            for pr in range(NPAIR):
                emit_qk_group(0, pr, True)
            for tb in range(NKB):
                emit_v_tb(tb)

            units = [(pr, J) for J in range(NCH) for pr in range(NPAIR)]
            glist = []
            for pr, J in units:
                for I in range(NKB * (J + 1)):
                    glist.append((pr, J, I, I == NKB * (J + 1) - 1))

            def win_start(Jw):
                g = 0
                for pr, J in units:
                    if J == Jw:
                        return g
                    g += NKB * (J + 1)
                return len(glist)

            gidx = 0
            unit_end = {}
            for pr, J in units:
                gidx += NKB * (J + 1)
                unit_end[(pr, J)] = gidx

            fillers = []
            for c in range(1, NCH):
                avail = win_start(c - 1)
                for pr in range(NPAIR):
                    fillers.append((avail, 4096, lambda c=c, pr=pr: emit_qk_group(c, pr, False)))
                    fillers.append((avail, 4096, lambda c=c, pr=pr: emit_qk_group(c, pr, True)))
                for tb in range(NKB * c, NKB * c + NKB):
                    fillers.append((avail, 2048, lambda tb=tb: emit_v_tb(tb)))
            for J in range(NCH - 1):  # last chunk's outs go in the tail
                avail = unit_end[(1, J)] + SKEW
                for tb in range(NKB * J, NKB * (J + 1)):
                    for ec in range(NEO):
                        fillers.append((avail, 1024, lambda tb=tb, ec=ec: emit_out(tb, ec)))
            fillers.sort(key=lambda f: f[0])

            ctx_of_unit = {}
            fi = 0
            budget = 0.0
            pending = []
            for gi, (pr, J, I, last) in enumerate(glist):
                if I == 0:
                    cx0 = ps_c0.tile([D + 1, CH], f32, tag="c0", name=f"cx0_{pr}{J}")
                    cx1 = ps_c1.tile([D + 1, CH], f32, tag="c1", name=f"cx1_{pr}{J}")
                    ctx_of_unit[(pr, J)] = (cx0, cx1)
                pT = emit_scores(pr, J, I)
                pending.append((pr, J, I, last, pT))
                if len(pending) > SKEW:
                    apr, aJ, aI, alast, apT = pending.pop(0)
                    cx0, cx1 = ctx_of_unit[(apr, aJ)]
                    emit_av(apr, aJ, aI, apT, cx0, cx1)
                    if alast:
                        emit_normalize(apr, aJ, cx0, cx1)
                        del ctx_of_unit[(apr, aJ)]
                # ~131k filler cycles over 80 groups
                budget = min(budget + 1650.0, 8192.0)
                while fi < len(fillers) and fillers[fi][0] <= gi and budget >= fillers[fi][1]:
                    budget -= fillers[fi][1]
                    fillers[fi][2]()
                    fi += 1
            while pending:
                apr, aJ, aI, alast, apT = pending.pop(0)
                cx0, cx1 = ctx_of_unit[(apr, aJ)]
                emit_av(apr, aJ, aI, apT, cx0, cx1)
                if alast:
                    emit_normalize(apr, aJ, cx0, cx1)
                    del ctx_of_unit[(apr, aJ)]
            while fi < len(fillers):
                fillers[fi][2]()
                fi += 1
            for tb in range(NTB - NKB, NTB):
                for ec in range(NEO):
                    emit_out(tb, ec)

    nc.compile()
    return nc


def get_nc():
    global _NC_CACHE
    if _NC_CACHE is None:
        _NC_CACHE = _build_nc()
    return _NC_CACHE


def make_in_maps(x, Wq, Wk, Wv, Wo):
    x = np.asarray(x, dtype=np.float32)
    Wq = np.asarray(Wq, dtype=np.float32)
    Wk = np.asarray(Wk, dtype=np.float32)
    Wv = np.asarray(Wv, dtype=np.float32)
    Wo = np.asarray(Wo, dtype=np.float32)
    b16 = ml_dtypes.bfloat16
    in_maps = []
    for cid in range(N_CORES):
        b, g = divmod(cid, TP)
        sl = slice(DL * g, DL * (g + 1))
        in_maps.append(
            {
                "xT": np.ascontiguousarray(x[b].T).astype(b16),
                "wqT": np.ascontiguousarray(Wq[sl].T).astype(b16),
                "wkT": np.ascontiguousarray(Wk[sl].T).astype(b16),
                "wvT": np.ascontiguousarray(Wv[sl].T).astype(b16),
                "woT": np.ascontiguousarray(Wo[:, sl].T).astype(b16),
            }
        )
    return in_maps


def _combine(results, bo):
    bo = np.asarray(bo, dtype=np.float32)
    y = np.zeros((B, S, E), dtype=np.float32)
    for c in range(N_CORES):
        y[c // TP] += np.asarray(results[c]["out"], dtype=np.float32)
    y += bo
    return y


def kernel(x, Wq, Wk, Wv, Wo, bo):
    nc = get_nc()
    in_maps = make_in_maps(x, Wq, Wk, Wv, Wo)
    res = run_bass_kernel_spmd(nc, in_maps, list(range(N_CORES)))
    return _combine(res.results, bo)


def kernel_traced(x, Wq, Wk, Wv, Wo, bo, trace_cores=None):
    """Like kernel() but with NTFF tracing; returns (output, BassKernelResults)."""
    nc = get_nc()
    in_maps = make_in_maps(x, Wq, Wk, Wv, Wo)
    res = run_bass_kernel_spmd(
        nc, in_maps, list(range(N_CORES)), trace=True, trace_cores=trace_cores
    )
    return _combine(res.results, bo), res
